# revision 8
# baseline (speedup 1.0000x reference)
"""Trainium2 Bass kernel for nn_Attend (decomposable attention).

Computation (reference):
    f_A = relu(relu(A @ W1 + b1) @ W2 + b2)      [b, m, h]
    f_B = relu(relu(B @ W1 + b1) @ W2 + b2)      [b, n, h]
    e = f_A @ f_B^T                               [b, m, n]
    beta  = softmax(e, axis=-1) @ B               [b, m, d]
    alpha = softmax(e, axis=-2)^T @ A             [b, n, d]
    returns (beta, alpha)

Shapes: b=4, m=n=4096, d=128, h=256. Scores e lie in ~[0.5, 8.3] so
exp() needs no max-subtraction; cross-shard softmax combines are plain
sums of partials done on the host (beta/alpha leave unnormalized with
their denominators).

Sharding: 8 cores = (batch, m-half). Each core handles 2048 m-rows of
one batch against all 4096 n. beta is local up to the row softmax
division; alpha is summed over the 2 cores of a batch on the host.

Key design points vs the 140us predecessor:
- E is computed with fp8e4 DoubleRow matmuls: f (the MLP output, used
  ONLY for E) is written as fp8e4 in the [128 hi, 2 ho, tok] interleaved
  layout straight from the MLP epilogue; one matmul then contracts all
  256 h at 0.5 cycles/row (end-to-end rel err 1.8e-2 vs the 2e-2 gate,
  dominated by the fp8 quantization of f; everything else rides bf16).
- exp runs 1024-wide (one ACT instruction per m-chunk x strip-pair),
  with no accum_out: both softmax denominators come from near-free PE
  matmuls against a ones vector (output free size 1 ~= 1 cycle):
  cols (sum over m) uses P chunks as stationary in native orientation,
  rows (sum over n) uses the xbar-transposed P^T blocks as stationary.
- PSUM (8 banks): et [128,1024] x2 = 4, alpha pair accumulator
  [128,1024] = 2, beta quarter accumulator [128,512] = 1 (flushed to a
  f32 SBUF accumulator by DVE adds once per (pair, m-quarter)), sums = 1.
- beta^T quarter (p,q) matmuls run INSIDE pair p, two per chunk slot
  starting once the four transposes they need have landed; quarters
  finishing after the pair spill into the next pair's slots (tail for
  the last). Finished beta quarters stream to DRAM immediately.
- All MLP tensors (inputs + weights) are bf16: halves the input DMA and
  keeps the PE at full rate; h1 bias-relu on ACT, h2 bias-relu(+fp8
  cast) on DVE.

Per-core budgets (cost model): PE ~201k cycles ~84us busy, ACT ~81us,
DMA pipe ~75us (57us of it the 16MB of P^T xbar transposes), DVE ~33us.
"""

import sys

import numpy as np

if "/opt/trn_rl_repo" not in sys.path:
    sys.path.insert(0, "/opt/trn_rl_repo")

import ml_dtypes  # noqa: E402

import concourse.bass as bass  # noqa: E402
import concourse.mybir as mybir  # noqa: E402
import concourse.tile as tile  # noqa: E402
from concourse import bacc  # noqa: E402

F32 = mybir.dt.float32
BF16 = mybir.dt.bfloat16
FP8 = mybir.dt.float8e4
EXP = mybir.ActivationFunctionType.Exp
COPY = mybir.ActivationFunctionType.Copy
RELU = mybir.ActivationFunctionType.Relu
DR = mybir.MatmulPerfMode.DoubleRow

D = 128      # model dim
H = 256      # hidden dim
M = 2048     # rows per core (half of 4096)
N = 4096     # full sequence
MC = M // 128   # m chunks per core (16)
NB = N // 128   # n blocks (32)
NS = N // 512   # n strips (8)
NP = NS // 2    # strip pairs (4)

_CACHE = {}


def _mlp_transposed(nc, pools, xT, f8, w1, w2, b1c, b2c, zero, seq):
    """f8[:, i, :] = relu(W2^T @ relu(W1^T @ xT + b1) + b2) in fp8e4.

    xT: [128 d, seq] bf16; f8: [128, 2, seq] fp8e4 (h split in 2 ko
    blocks). h1 bias+relu on ACT (bf16 out), h2 bias+relu on DVE with
    direct fp8 cast.
    """
    mlp_ps, h1_pool = pools
    h1 = [h1_pool.tile([128, seq], BF16, tag=f"h1_{i}", name=f"h1_{i}")
          for i in range(2)]
    nchunks = seq // 512

    def h2(s):
        sl = bass.ts(s, 512)
        for i in range(2):
            ps = mlp_ps.tile([128, 512], F32, tag="mlp")
            for kh in range(2):
                nc.tensor.matmul(ps, w2[:, bass.ts(kh * 2 + i, 128)],
                                 h1[kh][:, sl],
                                 start=(kh == 0), stop=(kh == 1))
            nc.vector.tensor_scalar(
                out=f8[:, i, sl], in0=ps,
                scalar1=b2c[:, i:i + 1], scalar2=zero,
                op0=mybir.AluOpType.add, op1=mybir.AluOpType.max)

    for s in range(nchunks):
        sl = bass.ts(s, 512)
        for i in range(2):
            ps = mlp_ps.tile([128, 512], F32, tag="mlp")
            nc.tensor.matmul(ps, w1[:, bass.ts(i, 128)], xT[:, sl],
                             start=True, stop=True)
            nc.scalar.activation(h1[i][:, sl], ps, RELU,
                                 bias=b1c[:, i:i + 1])
        if s > 0:
            h2(s - 1)
    h2(nchunks - 1)


def _build():
    """Build + compile the per-core Bass program (same NEFF on all 8 cores)."""
    nc = bacc.Bacc(None, target_bir_lowering=False)

    # inputs
    atb = nc.declare_dram_parameter("atb", [128, M], BF16, isOutput=False)
    btb = nc.declare_dram_parameter("btb", [128, N], BF16, isOutput=False)
    anr = nc.declare_dram_parameter("anr", [128, M], BF16, isOutput=False)
    bnr = nc.declare_dram_parameter("bnr", [128, N], BF16, isOutput=False)
    w1 = nc.declare_dram_parameter("w1", [128, H], BF16, isOutput=False)
    w2 = nc.declare_dram_parameter("w2", [128, 2 * H], BF16, isOutput=False)
    b1 = nc.declare_dram_parameter("b1", [128, 2], F32, isOutput=False)
    b2 = nc.declare_dram_parameter("b2", [128, 2], F32, isOutput=False)
    # outputs (beta^T and alpha^T unnormalized; host divides by the sums)
    betat_d = nc.declare_dram_parameter("betat", [128, M], F32, isOutput=True)
    rows_d = nc.declare_dram_parameter("rows", [128, MC], F32, isOutput=True)
    alphat_d = nc.declare_dram_parameter("alphat", [128, N], F32,
                                         isOutput=True)
    cols_d = nc.declare_dram_parameter("cols", [128, NB], F32, isOutput=True)

    with tile.TileContext(nc) as tc, \
         tc.tile_pool(name="const", bufs=1) as const:
        # persistent SBUF tensors
        w1_sb = const.tile([128, H], BF16, tag="w1")
        w2_sb = const.tile([128, 2 * H], BF16, tag="w2")
        b1_sb = const.tile([128, 2], F32, tag="b1")
        b2_sb = const.tile([128, 2], F32, tag="b2")
        anr_sb = const.tile([128, M], BF16, tag="anr")
        bnr_sb = const.tile([128, N], BF16, tag="bnr")

        zero = const.tile([128, 1], F32, tag="zero")
        nc.vector.memset(zero, 0.0)
        ones = const.tile([128, 1], BF16, tag="ones")
        nc.vector.memset(ones, 1.0)
        # trigger the exp table-set load on ACT immediately (overlaps with
        # the input DMAs instead of stalling the first real exp)
        dummy = const.tile([128, 1], F32, tag="dummy")
        nc.scalar.activation(dummy, zero, EXP)

        f8at = const.tile([128, 2, M], FP8, tag="f8at")
        f8bt = const.tile([128, 2, N], FP8, tag="f8bt")
        beta_sb = const.tile([128, M], F32, tag="beta_sb")
        rows_sb = const.tile([128, MC], F32, tag="rows_sb")
        cols_sb = const.tile([128, NB], F32, tag="cols_sb")

        # ---- phase 1: MLPs (atb/btb live in a pool that closes after) ----
        with tc.tile_pool(name="mlp_in", bufs=1) as mlp_in, \
             tc.tile_pool(name="mlp_ps", bufs=4, space="PSUM") as mlp_ps, \
             tc.tile_pool(name="h1", bufs=2) as h1_pool:
            atb_sb = mlp_in.tile([128, M], BF16, tag="atb")
            btb_sb = mlp_in.tile([128, N], BF16, tag="btb")
            # DMA issue is expensive and transfers drain through a mostly-
            # serial pipe: first operands first, ordered by first use.
            nc.gpsimd.dma_start(atb_sb[:, 0:512], atb[:, 0:512])
            nc.sync.dma_start(w1_sb, w1[:])
            nc.sync.dma_start(b1_sb, b1[:])
            nc.sync.dma_start(w2_sb, w2[:])
            nc.sync.dma_start(b2_sb, b2[:])
            nc.gpsimd.dma_start(atb_sb[:, 512:2048], atb[:, 512:2048])
            nc.gpsimd.dma_start(btb_sb[:, 0:2048], btb[:, 0:2048])
            nc.gpsimd.dma_start(btb_sb[:, 2048:4096], btb[:, 2048:4096])
            nc.gpsimd.dma_start(anr_sb, anr[:])
            nc.gpsimd.dma_start(bnr_sb, bnr[:])

            # A's MLP first: the first E chunk is gated by f8at chunk 0 and
            # f8bt's first strips.
            pools = (mlp_ps, h1_pool)
            _mlp_transposed(nc, pools, atb_sb, f8at,
                            w1_sb, w2_sb, b1_sb, b2_sb, zero, M)
            _mlp_transposed(nc, pools, btb_sb, f8bt,
                            w1_sb, w2_sb, b1_sb, b2_sb, zero, N)

        # ---- phase 2: fused E/exp/sums/alpha/beta main loop ----
        # Per pair p (1024 n), per chunk c: one et [128,1024] (2 psum
        # banks), two fp8 DoubleRow matmuls, one 1024-wide exp -> P chunk
        # (bf16), 8 tiny cols matmuls (P slices stationary, ones moving),
        # 2 alpha matmuls, one xbar transpose into pt, 8 tiny rows
        # matmuls (lagged, pt slices stationary), and 2 beta matmuls of
        # the most recent ready quarter.
        with tc.tile_pool(name="et_ps", bufs=2, space="PSUM") as et_pool, \
             tc.tile_pool(name="al_ps", bufs=1, space="PSUM") as al_pool, \
             tc.tile_pool(name="bt_ps", bufs=1, space="PSUM") as bt_pool, \
             tc.tile_pool(name="sm_ps", bufs=1, space="PSUM") as sm_pool, \
             tc.tile_pool(name="pp", bufs=1) as ppool, \
             tc.tile_pool(name="pt", bufs=2) as ptpool, \
             tc.tile_pool(name="stage", bufs=2) as stage:
            sums_ps = sm_pool.tile([128, 512], F32, tag="sums",
                                   name="sums_ps")
            # PSUM start=True zeroes has_written for the WHOLE 2KB bank
            # (ZERO_REGION_SIZE), so interleaved per-column accumulation
            # groups cannot each carry their own start. Instead: one
            # clearing matmul marks the entire sums bank pending-zero
            # (its value lands in an unused scratch column), and every
            # rows/cols matmul runs start=False — the first write per
            # element overwrites, later ones accumulate.
            nc.tensor.matmul(sums_ps[:, 511:512], anr_sb[:, 0:128], ones,
                             start=True, stop=True, skip_group_check=True)

            # beta work queue: thunks consumed two per chunk slot
            beta_q = []
            bt_tiles = {}

            def make_beta(p, q, pt_t):
                bt_ps = bt_pool.tile([128, 512], F32, tag="bt",
                                     name=f"bt{p}_{q}")
                bt_tiles[(p, q)] = bt_ps

                def mm(j, q=q, p=p):
                    nc.tensor.matmul(
                        bt_ps, bnr_sb[:, bass.ts(p * 8 + j, 128)],
                        pt_t[:, j, bass.ts(q, 512)],
                        start=(j == 0), stop=(j == 7))

                def flush(q=q, p=p):
                    # fold the quarter into the SBUF accumulator; stream
                    # the finished quarter out after the last pair
                    qsl = bass.ts(q, 512)
                    if p == 0:
                        nc.vector.tensor_copy(beta_sb[:, qsl], bt_ps)
                    else:
                        nc.vector.tensor_tensor(
                            out=beta_sb[:, qsl], in0=bt_ps,
                            in1=beta_sb[:, qsl], op=mybir.AluOpType.add)
                    if p == NP - 1:
                        nc.gpsimd.dma_start(betat_d[:, qsl],
                                            beta_sb[:, qsl])

                return [(lambda j=j: mm(j)) for j in range(8)] + [flush]

            rows_q = []   # deferred rows thunks (wait on transposes)

            def make_rows(p, c, pt_t):
                def mm(j, c=c, p=p):
                    nc.tensor.matmul(
                        sums_ps[:, c:c + 1],
                        pt_t[:, j, bass.ts(c, 128)], ones,
                        start=False, stop=False,
                        skip_group_check=True)

                def emit():
                    for j in range(8):
                        mm(j)
                return emit

            for p in range(NP):
                pt_t = ptpool.tile([128, 8, M], BF16, tag="pt",
                                   name=f"pt{p}")
                p_tiles = [ppool.tile([128, 1024], BF16, tag=f"P{c}",
                                      name=f"P{c}_{p}") for c in range(MC)]
                al = al_pool.tile([128, 1024], F32, tag="al", name=f"al{p}")
                for c in range(MC):
                    et = et_pool.tile([128, 1024], F32, tag="et")
                    p_c = p_tiles[c]
                    # E: two fp8 DoubleRow matmuls (contract all 256 h)
                    for half in range(2):
                        nc.tensor.matmul(
                            et[:, bass.ts(half, 512)],
                            f8at[:, :, bass.ts(c, 128)],
                            f8bt[:, :, bass.ts(2 * p + half, 512)],
                            start=True, stop=True, perf_mode=DR)
                    # exp (1024 wide)
                    nc.scalar.activation(p_c, et, EXP)
                    # beta quarter work: three thunks per slot once ready
                    # (production is 9 thunks per quarter, 36 per pair;
                    # 3/slot over c=4..15 keeps the queue bounded with one
                    # quarter spilling into the next pair's early slots).
                    # Emitted BEFORE alpha(c-1) so the PE has ready work
                    # while exp(c-1) drains.
                    if c >= 4:
                        for _ in range(3):
                            if beta_q:
                                beta_q.pop(0)()
                    # previous chunk's alpha + cols (P-gated, lag 1 so the
                    # PE stream doesn't park on the exp)
                    if c > 0:
                        _alpha_cols(nc, al, anr_sb, p_tiles[c - 1], c - 1,
                                    p, sums_ps, ones)
                    # transpose the completed P chunk
                    nc.sync.dma_start_transpose(
                        pt_t[:, :, bass.ts(c, 128)], p_c)
                    if c % 4 == 3:
                        beta_q.extend(make_beta(p, c // 4, pt_t))
                    # rows for chunk c-4 (transpose has landed by now)
                    rows_q.append(make_rows(p, c, pt_t))
                    if len(rows_q) > 4:
                        rows_q.pop(0)()
                # last chunk's alpha + cols, then evacuate the pair's alpha
                _alpha_cols(nc, al, anr_sb, p_tiles[MC - 1], MC - 1,
                            p, sums_ps, ones)
                a_sb = stage.tile([128, 1024], F32, tag="as", name=f"as{p}")
                nc.vector.tensor_copy(a_sb, al)
                nc.gpsimd.dma_start(alphat_d[:, bass.ts(p, 1024)], a_sb)

            # ---- tail: drain beta + rows queues, evacuate sums ----
            for fn in beta_q:
                fn()
            for fn in rows_q:
                fn()
            nc.vector.tensor_copy(rows_sb, sums_ps[:, 0:MC])
            nc.vector.tensor_copy(cols_sb, sums_ps[:, 16:16 + NB])
            nc.sync.dma_start(rows_d[:], rows_sb)
            nc.sync.dma_start(cols_d[:], cols_sb)

    nc.compile()
    return nc


def _alpha_cols(nc, al, anr_sb, p_c, c, p, sums_ps, ones):
    """Alpha accumulation + cols sums for chunk c of pair p."""
    for half in range(2):
        nc.tensor.matmul(al[:, bass.ts(half, 512)],
                         anr_sb[:, bass.ts(c, 128)],
                         p_c[:, bass.ts(half, 512)],
                         start=(c == 0), stop=(c == MC - 1))
    # cols: sums_ps[:, 16 + jg] += P_c[:, jg-block]^T @ ones
    for half in range(2):
        for j4 in range(4):
            jg = (2 * p + half) * 4 + j4
            nc.tensor.matmul(
                sums_ps[:, 16 + jg:17 + jg],
                p_c[:, bass.ts(half * 4 + j4, 128)], ones,
                start=False, stop=False,
                skip_group_check=True)


def _get_nc():
    if "nc" not in _CACHE:
        _CACHE["nc"] = _build()
    return _CACHE["nc"]


def _get_runner():
    """Jitted 8-core shard_map executor built once (mirrors
    bass2jax.run_bass_via_pjrt, but cacheable across calls)."""
    if "runner" in _CACHE:
        return _CACHE["runner"]
    import jax
    from jax.sharding import Mesh, PartitionSpec
    from jax.experimental.shard_map import shard_map
    import concourse.mybir as mb
    from concourse.bass2jax import (
        _bass_exec_p, install_neuronx_cc_hook, partition_id_tensor)

    nc = _get_nc()
    install_neuronx_cc_hook()

    in_names, out_names, out_avals = [], [], []
    partition_name = (nc.partition_id_tensor.name
                      if nc.partition_id_tensor else None)
    for alloc in nc.m.functions[0].allocations:
        if not isinstance(alloc, mb.MemoryLocationSet):
            continue
        name = alloc.memorylocations[0].name
        if alloc.kind == "ExternalInput":
            if name != partition_name:
                in_names.append(name)
        elif alloc.kind == "ExternalOutput":
            out_names.append(name)
            out_avals.append(jax.core.ShapedArray(
                tuple(alloc.tensor_shape), mb.dt.np(alloc.dtype)))
    n_params = len(in_names)
    zero_outs = [np.zeros((8 * a.shape[0], *a.shape[1:]), a.dtype)
                 for a in out_avals]
    all_in_names = in_names + out_names
    if partition_name is not None:
        all_in_names = all_in_names + [partition_name]

    def _body(*args):
        operands = list(args)
        if partition_name is not None:
            operands.append(partition_id_tensor())
        return tuple(_bass_exec_p.bind(
            *operands,
            out_avals=tuple(out_avals),
            in_names=tuple(all_in_names),
            out_names=tuple(out_names),
            lowering_input_output_aliases=(),
            sim_require_finite=True,
            sim_require_nnan=True,
            nc=nc,
        ))

    devices = jax.devices()[:8]
    mesh = Mesh(np.asarray(devices), ("core",))
    nin = n_params + len(out_names)
    sharded = jax.jit(shard_map(
        _body, mesh=mesh,
        in_specs=(PartitionSpec("core"),) * nin,
        out_specs=(PartitionSpec("core"),) * len(out_names),
        check_rep=False))
    zeros_dev = [jax.device_put(z) for z in zero_outs]
    _CACHE["runner"] = (sharded, in_names, out_names, out_avals, zeros_dev)
    return _CACHE["runner"]


def run_cores(in_maps):
    """Run the 8-core program; returns list of per-core output dicts."""
    sharded, in_names, out_names, out_avals, zeros_dev = _get_runner()
    concat_in = [np.concatenate([m[name] for m in in_maps], axis=0)
                 for name in in_names]
    out_arrs = sharded(*concat_in, *zeros_dev)
    out_arrs = [np.asarray(o) for o in out_arrs]
    return [
        {name: out_arrs[i].reshape(8, *out_avals[i].shape)[c]
         for i, name in enumerate(out_names)}
        for c in range(8)
    ]


def build_in_maps(A, B, W1, b1, W2, b2):
    A = np.ascontiguousarray(np.asarray(A, dtype=np.float32))
    B = np.ascontiguousarray(np.asarray(B, dtype=np.float32))
    W1 = np.asarray(W1, dtype=np.float32)
    b1 = np.asarray(b1, dtype=np.float32)
    W2 = np.asarray(W2, dtype=np.float32)
    b2 = np.asarray(b2, dtype=np.float32)
    nbatch, seq, d = A.shape
    assert (nbatch, seq, d) == (4, N, D), (nbatch, seq, d)

    bf = ml_dtypes.bfloat16
    w1r = np.ascontiguousarray(W1).astype(bf)                     # [128, 256]
    w2r = np.ascontiguousarray(
        W2.reshape(2, 128, 2, 128).transpose(1, 0, 2, 3).reshape(128, 512)
    ).astype(bf)
    b1c = np.ascontiguousarray(b1.reshape(2, 128).T)              # [128, 2]
    b2c = np.ascontiguousarray(b2.reshape(2, 128).T)

    in_maps = []
    for core in range(8):
        b_i, half = divmod(core, 2)
        Ah = A[b_i, half * M:(half + 1) * M]                      # [2048, 128]
        Bf = B[b_i]                                               # [4096, 128]
        in_maps.append({
            "atb": np.ascontiguousarray(Ah.T).astype(bf),
            "btb": np.ascontiguousarray(Bf.T).astype(bf),
            "anr": np.ascontiguousarray(
                Ah.reshape(MC, 128, 128).transpose(1, 0, 2).reshape(128, M)
            ).astype(bf),
            "bnr": np.ascontiguousarray(
                Bf.reshape(NB, 128, 128).transpose(1, 0, 2).reshape(128, N)
            ).astype(bf),
            "w1": w1r, "w2": w2r, "b1": b1c, "b2": b2c,
        })
    return in_maps


def kernel(A, B, W1, b1, W2, b2):
    in_maps = build_in_maps(A, B, W1, b1, W2, b2)
    results = run_cores(in_maps)

    beta = np.empty((4, N, D), dtype=np.float32)
    alpha = np.empty((4, N, D), dtype=np.float32)
    for b_i in range(4):
        r0 = results[2 * b_i]
        r1 = results[2 * b_i + 1]
        for half, r in ((0, r0), (1, r1)):
            # rows[p, c] is the rowsum for m = c*128 + p
            rowv = r["rows"].T.reshape(1, M)
            beta[b_i, half * M:(half + 1) * M] = (r["betat"] / rowv).T
        num = r0["alphat"] + r1["alphat"]                          # [128, 4096]
        csum = r0["cols"] + r1["cols"]                             # [128, 32]
        # csum[p, j] corresponds to n = j*128 + p
        alpha[b_i] = (num / csum.T.reshape(1, N)).T.reshape(N, D)
    return beta, alpha


if __name__ == "__main__":
    rng = np.random.default_rng(0)
    A = rng.standard_normal((4, N, D)).astype(np.float32)
    B = rng.standard_normal((4, N, D)).astype(np.float32)
    s1, s2 = 1.0 / np.sqrt(D), 1.0 / np.sqrt(H)
    W1 = rng.uniform(-s1, s1, (D, H)).astype(np.float32)
    b1 = rng.uniform(-s1, s1, H).astype(np.float32)
    W2 = rng.uniform(-s2, s2, (H, H)).astype(np.float32)
    b2 = rng.uniform(-s2, s2, H).astype(np.float32)
    beta, alpha = kernel(A=A, B=B, W1=W1, b1=b1, W2=W2, b2=b2)
    print("beta", beta.shape, "alpha", alpha.shape)


# revision 16
# speedup vs baseline: 1.0859x; 1.0859x over previous
"""Trainium2 Bass kernel for nn_Attend (decomposable attention).

Computation (reference):
    f_A = relu(relu(A @ W1 + b1) @ W2 + b2)      [b, m, h]
    f_B = relu(relu(B @ W1 + b1) @ W2 + b2)      [b, n, h]
    e = f_A @ f_B^T                               [b, m, n]
    beta  = softmax(e, axis=-1) @ B               [b, m, d]
    alpha = softmax(e, axis=-2)^T @ A             [b, n, d]
    returns (beta, alpha)

Shapes: b=4, m=n=4096, d=128, h=256. Scores e lie in ~[0.5, 8.3] so
exp() needs no max-subtraction; cross-shard softmax combines are plain
sums of partials done on the host (beta/alpha leave unnormalized with
their denominators).

Sharding: 8 cores = (batch, m-half). Each core handles 2048 m-rows of
one batch against all 4096 n. beta is local up to the row softmax
division; alpha is summed over the 2 cores of a batch on the host.

Key design points vs the 140us predecessor:
- E is computed with fp8e4 DoubleRow matmuls: f (the MLP output, used
  ONLY for E) is written as fp8e4 in the [128 hi, 2 ho, tok] interleaved
  layout straight from the MLP epilogue; one matmul then contracts all
  256 h at 0.5 cycles/row (end-to-end rel err 1.8e-2 vs the 2e-2 gate,
  dominated by the fp8 quantization of f; everything else rides bf16).
- exp runs 1024-wide (one ACT instruction per m-chunk x strip-pair),
  with no accum_out: both softmax denominators come from near-free PE
  matmuls against a ones vector (output free size 1 ~= 1 cycle):
  cols (sum over m) uses P chunks as stationary in native orientation,
  rows (sum over n) uses the xbar-transposed P^T blocks as stationary.
- PSUM (8 banks): et [128,1024] x2 = 4, alpha pair accumulator
  [128,1024] = 2, beta quarter accumulator [128,512] = 1 (flushed to a
  f32 SBUF accumulator by DVE adds once per (pair, m-quarter)), sums = 1.
- beta^T quarter (p,q) matmuls run INSIDE pair p, two per chunk slot
  starting once the four transposes they need have landed; quarters
  finishing after the pair spill into the next pair's slots (tail for
  the last). Finished beta quarters stream to DRAM immediately.
- All MLP tensors (inputs + weights) are bf16: halves the input DMA and
  keeps the PE at full rate; h1 bias-relu on ACT, h2 bias-relu(+fp8
  cast) on DVE.

Per-core budgets (cost model): PE ~201k cycles ~84us busy, ACT ~81us,
DMA pipe ~75us (57us of it the 16MB of P^T xbar transposes), DVE ~33us.
"""

import sys

import numpy as np

if "/opt/trn_rl_repo" not in sys.path:
    sys.path.insert(0, "/opt/trn_rl_repo")

import ml_dtypes  # noqa: E402

import concourse.bass as bass  # noqa: E402
import concourse.mybir as mybir  # noqa: E402
import concourse.tile as tile  # noqa: E402
from concourse import bacc  # noqa: E402

F32 = mybir.dt.float32
BF16 = mybir.dt.bfloat16
FP8 = mybir.dt.float8e4
EXP = mybir.ActivationFunctionType.Exp
COPY = mybir.ActivationFunctionType.Copy
RELU = mybir.ActivationFunctionType.Relu
DR = mybir.MatmulPerfMode.DoubleRow

D = 128      # model dim
H = 256      # hidden dim
M = 2048     # rows per core (half of 4096)
N = 4096     # full sequence
MC = M // 128   # m chunks per core (16)
NB = N // 128   # n blocks (32)
NS = N // 512   # n strips (8)
NP = NS // 2    # strip pairs (4)

_CACHE = {}


def _mlp_transposed(nc, pools, xT, f8, w1, w2, b1c, b2c, zero, seq):
    """f8[:, i, :] = relu(W2^T @ relu(W1^T @ xT + b1) + b2) in fp8e4.

    xT: [128 d, seq] bf16; f8: [128, 2, seq] fp8e4 (h split in 2 ko
    blocks). h1 bias+relu on ACT (bf16 out), h2 bias+relu on DVE with
    direct fp8 cast.
    """
    mlp_ps, h1_pool = pools
    h1 = [h1_pool.tile([128, seq], BF16, tag=f"h1_{i}", name=f"h1_{i}")
          for i in range(2)]
    nchunks = seq // 512

    def h2(s):
        sl = bass.ts(s, 512)
        for i in range(2):
            ps = mlp_ps.tile([128, 512], F32, tag="mlp")
            for kh in range(2):
                nc.tensor.matmul(ps, w2[:, bass.ts(kh * 2 + i, 128)],
                                 h1[kh][:, sl],
                                 start=(kh == 0), stop=(kh == 1))
            nc.vector.tensor_scalar(
                out=f8[:, i, sl], in0=ps,
                scalar1=b2c[:, i:i + 1], scalar2=zero,
                op0=mybir.AluOpType.add, op1=mybir.AluOpType.max)

    for s in range(nchunks):
        sl = bass.ts(s, 512)
        for i in range(2):
            ps = mlp_ps.tile([128, 512], F32, tag="mlp")
            nc.tensor.matmul(ps, w1[:, bass.ts(i, 128)], xT[:, sl],
                             start=True, stop=True)
            nc.scalar.activation(h1[i][:, sl], ps, RELU,
                                 bias=b1c[:, i:i + 1])
        if s > 0:
            h2(s - 1)
    h2(nchunks - 1)


def _build():
    """Build + compile the per-core Bass program (same NEFF on all 8 cores)."""
    nc = bacc.Bacc(None, target_bir_lowering=False)

    # inputs
    atb = nc.declare_dram_parameter("atb", [128, M], BF16, isOutput=False)
    btb = nc.declare_dram_parameter("btb", [128, N], BF16, isOutput=False)
    anr = nc.declare_dram_parameter("anr", [128, M], BF16, isOutput=False)
    bnr = nc.declare_dram_parameter("bnr", [128, N], BF16, isOutput=False)
    w1 = nc.declare_dram_parameter("w1", [128, H], BF16, isOutput=False)
    w2 = nc.declare_dram_parameter("w2", [128, 2 * H], BF16, isOutput=False)
    b1 = nc.declare_dram_parameter("b1", [128, 2], F32, isOutput=False)
    b2 = nc.declare_dram_parameter("b2", [128, 2], F32, isOutput=False)
    # outputs (beta^T and alpha^T unnormalized; host divides by the sums)
    betat_d = nc.declare_dram_parameter("betat", [128, M], F32, isOutput=True)
    rows_d = nc.declare_dram_parameter("rows", [128, MC], F32, isOutput=True)
    alphat_d = nc.declare_dram_parameter("alphat", [128, N], F32,
                                         isOutput=True)
    cols_d = nc.declare_dram_parameter("cols", [128, NB], F32, isOutput=True)

    with tile.TileContext(nc) as tc, \
         tc.tile_pool(name="const", bufs=1) as const:
        # persistent SBUF tensors
        w1_sb = const.tile([128, H], BF16, tag="w1")
        w2_sb = const.tile([128, 2 * H], BF16, tag="w2")
        b1_sb = const.tile([128, 2], F32, tag="b1")
        b2_sb = const.tile([128, 2], F32, tag="b2")
        anr_sb = const.tile([128, M], BF16, tag="anr")
        bnr_sb = const.tile([128, N], BF16, tag="bnr")

        zero = const.tile([128, 1], F32, tag="zero")
        nc.vector.memset(zero, 0.0)
        ones = const.tile([128, 1], BF16, tag="ones")
        nc.vector.memset(ones, 1.0)
        # trigger the exp table-set load on ACT immediately (overlaps with
        # the input DMAs instead of stalling the first real exp)
        dummy = const.tile([128, 1], F32, tag="dummy")
        nc.scalar.activation(dummy, zero, EXP)

        f8at = const.tile([128, 2, M], FP8, tag="f8at")
        f8bt = const.tile([128, 2, N], FP8, tag="f8bt")
        beta_sb = const.tile([128, M], F32, tag="beta_sb")
        rows_sb = const.tile([128, MC], F32, tag="rows_sb")
        cols_sb = const.tile([128, NB], F32, tag="cols_sb")

        # ---- phase 1: MLPs (atb/btb live in a pool that closes after) ----
        with tc.tile_pool(name="mlp_in", bufs=1) as mlp_in, \
             tc.tile_pool(name="mlp_ps", bufs=4, space="PSUM") as mlp_ps, \
             tc.tile_pool(name="h1", bufs=2) as h1_pool:
            atb_sb = mlp_in.tile([128, M], BF16, tag="atb")
            btb_sb = mlp_in.tile([128, N], BF16, tag="btb")
            # DMA issue is expensive and transfers drain through a mostly-
            # serial pipe: first operands first, ordered by first use.
            nc.gpsimd.dma_start(atb_sb[:, 0:512], atb[:, 0:512])
            nc.sync.dma_start(w1_sb, w1[:])
            nc.sync.dma_start(b1_sb, b1[:])
            nc.sync.dma_start(w2_sb, w2[:])
            nc.sync.dma_start(b2_sb, b2[:])
            nc.gpsimd.dma_start(atb_sb[:, 512:2048], atb[:, 512:2048])
            nc.gpsimd.dma_start(btb_sb[:, 0:2048], btb[:, 0:2048])
            nc.gpsimd.dma_start(btb_sb[:, 2048:4096], btb[:, 2048:4096])
            nc.gpsimd.dma_start(anr_sb, anr[:])
            nc.gpsimd.dma_start(bnr_sb, bnr[:])

            # A's MLP first: the first E chunk is gated by f8at chunk 0 and
            # f8bt's first strips.
            pools = (mlp_ps, h1_pool)
            _mlp_transposed(nc, pools, atb_sb, f8at,
                            w1_sb, w2_sb, b1_sb, b2_sb, zero, M)
            _mlp_transposed(nc, pools, btb_sb, f8bt,
                            w1_sb, w2_sb, b1_sb, b2_sb, zero, N)

        # ---- phase 2: fused E/exp/sums/alpha/beta main loop ----
        # Per pair p (1024 n), per chunk c: one et [128,1024] (2 psum
        # banks), two fp8 DoubleRow matmuls, one 1024-wide exp -> P chunk
        # (bf16), 8 tiny cols matmuls (P slices stationary, ones moving),
        # 2 alpha matmuls, one xbar transpose into pt, 8 tiny rows
        # matmuls (lagged, pt slices stationary), and 2 beta matmuls of
        # the most recent ready quarter.
        with tc.tile_pool(name="et_ps", bufs=2, space="PSUM") as et_pool, \
             tc.tile_pool(name="al_ps", bufs=1, space="PSUM") as al_pool, \
             tc.tile_pool(name="bt_ps", bufs=1, space="PSUM") as bt_pool, \
             tc.tile_pool(name="sm_ps", bufs=1, space="PSUM") as sm_pool, \
             tc.tile_pool(name="pp", bufs=1) as ppool, \
             tc.tile_pool(name="pt", bufs=2) as ptpool, \
             tc.tile_pool(name="stage", bufs=2) as stage:
            sums_ps = sm_pool.tile([128, 512], F32, tag="sums",
                                   name="sums_ps")
            # PSUM start=True zeroes has_written for the WHOLE 2KB bank
            # (ZERO_REGION_SIZE), so interleaved per-column accumulation
            # groups cannot each carry their own start. Instead: one
            # clearing matmul marks the entire sums bank pending-zero
            # (its value lands in an unused scratch column), and every
            # rows/cols matmul runs start=False — the first write per
            # element overwrites, later ones accumulate.
            nc.tensor.matmul(sums_ps[:, 511:512], anr_sb[:, 0:128], ones,
                             start=True, stop=True, skip_group_check=True)

            # beta work queue: (ready_slot, thunk) consumed up to 3 per
            # chunk slot, gated on the quarter's transpose having landed
            # (issue at slot 4q+3, ~5us HWDGE+transfer+sem latency)
            beta_q = []
            bt_tiles = {}

            def make_beta(p, q, pt_cm):
                bt_ps = bt_pool.tile([128, 512], F32, tag="bt",
                                     name=f"bt{p}_{q}")
                bt_tiles[(p, q)] = bt_ps

                def mm(j, q=q, p=p):
                    nc.tensor.matmul(
                        bt_ps, bnr_sb[:, bass.ts(p * 8 + j, 128)],
                        pt_cm[:, 4 * q:4 * q + 4, j, :],
                        start=(j == 0), stop=(j == 7))

                def flush(q=q, p=p):
                    # fold the quarter into the SBUF accumulator; stream
                    # the finished quarter out after the last pair
                    qsl = bass.ts(q, 512)
                    if p == 0:
                        nc.vector.tensor_copy(beta_sb[:, qsl], bt_ps)
                    else:
                        nc.vector.tensor_tensor(
                            out=beta_sb[:, qsl], in0=bt_ps,
                            in1=beta_sb[:, qsl], op=mybir.AluOpType.add)
                    if p == NP - 1:
                        nc.gpsimd.dma_start(betat_d[:, qsl],
                                            beta_sb[:, qsl])

                return [(lambda j=j: mm(j)) for j in range(8)] + [flush]

            rows_q = []   # deferred rows thunks (run one FULL PAIR later so
            # the transposes they read have landed long ago — parking >4
            # waiting instructions wedges the whole PE stream)

            def make_rows(p, c, pt_cm):
                def mm(j, c=c, p=p):
                    nc.tensor.matmul(
                        sums_ps[:, c:c + 1],
                        pt_cm[:, c, j, :], ones,
                        start=False, stop=False,
                        skip_group_check=True)

                def emit():
                    for j in range(8):
                        mm(j)
                return emit

            for p in range(NP):
                # pt is chunk-major: pt_cm[nq, c, j, mq] = P^T for
                # n = j*128+nq of this pair, m = c*128+mq. One [128, 4096]
                # xbar transpose covers FOUR chunks (the [c_l, j] block walk
                # of the destination matches the source column order).
                pt_cm = ptpool.tile([128, MC, 8, 128], BF16, tag="pt",
                                    name=f"pt{p}")
                p_mega = ppool.tile([128, MC * 1024], BF16, tag="P",
                                    name=f"P_{p}")
                al = al_pool.tile([128, 1024], F32, tag="al", name=f"al{p}")
                for c in range(MC):
                    et = et_pool.tile([128, 1024], F32, tag="et")
                    p_c = p_mega[:, c * 1024:(c + 1) * 1024]
                    # E: two fp8 DoubleRow matmuls (contract all 256 h)
                    for half in range(2):
                        nc.tensor.matmul(
                            et[:, bass.ts(half, 512)],
                            f8at[:, :, bass.ts(c, 128)],
                            f8bt[:, :, bass.ts(2 * p + half, 512)],
                            start=True, stop=True, perf_mode=DR)
                    # exp (1024 wide)
                    nc.scalar.activation(p_c, et, EXP)
                    # beta quarter work (before alpha(c-1) so the PE has
                    # ready work while exp(c-1) drains)
                    g = p * MC + c
                    n_pop = 0
                    while beta_q and beta_q[0][0] <= g and n_pop < 3:
                        beta_q.pop(0)[1]()
                        n_pop += 1
                    # previous chunk's alpha + cols (P-gated, lag 1 so the
                    # PE stream doesn't park on the exp)
                    if c > 0:
                        _alpha_cols(nc, al, anr_sb,
                                    p_mega[:, (c - 1) * 1024:c * 1024],
                                    c - 1, p, sums_ps, ones)
                    # one transpose per 4 completed P chunks
                    if c % 4 == 3:
                        k = c // 4
                        nc.sync.dma_start_transpose(
                            pt_cm[:, 4 * k:4 * k + 4, :, :],
                            p_mega[:, k * 4096:(k + 1) * 4096])
                        ready = p * MC + 4 * k + 9
                        beta_q.extend((ready, fn)
                                      for fn in make_beta(p, k, pt_cm))
                    # rows for this chunk, deferred one full pair
                    rows_q.append(make_rows(p, c, pt_cm))
                    if len(rows_q) > MC:
                        rows_q.pop(0)()
                # last chunk's alpha + cols, then evacuate the pair's alpha
                _alpha_cols(nc, al, anr_sb,
                            p_mega[:, (MC - 1) * 1024:MC * 1024],
                            MC - 1, p, sums_ps, ones)
                a_sb = stage.tile([128, 1024], F32, tag="as", name=f"as{p}")
                nc.vector.tensor_copy(a_sb, al)
                nc.gpsimd.dma_start(alphat_d[:, bass.ts(p, 1024)], a_sb)

            # ---- tail: drain beta + rows queues, evacuate sums ----
            for _, fn in beta_q:
                fn()
            for fn in rows_q:
                fn()
            nc.vector.tensor_copy(rows_sb, sums_ps[:, 0:MC])
            nc.vector.tensor_copy(cols_sb, sums_ps[:, 16:16 + NB])
            nc.sync.dma_start(rows_d[:], rows_sb)
            nc.sync.dma_start(cols_d[:], cols_sb)

    nc.compile()
    return nc


def _alpha_cols(nc, al, anr_sb, p_c, c, p, sums_ps, ones):
    """Alpha accumulation + cols sums for chunk c of pair p."""
    for half in range(2):
        nc.tensor.matmul(al[:, bass.ts(half, 512)],
                         anr_sb[:, bass.ts(c, 128)],
                         p_c[:, bass.ts(half, 512)],
                         start=(c == 0), stop=(c == MC - 1))
    # cols: sums_ps[:, 16 + jg] += P_c[:, jg-block]^T @ ones
    for half in range(2):
        for j4 in range(4):
            jg = (2 * p + half) * 4 + j4
            nc.tensor.matmul(
                sums_ps[:, 16 + jg:17 + jg],
                p_c[:, bass.ts(half * 4 + j4, 128)], ones,
                start=False, stop=False,
                skip_group_check=True)


def _get_nc():
    if "nc" not in _CACHE:
        _CACHE["nc"] = _build()
    return _CACHE["nc"]


def _get_runner():
    """Jitted 8-core shard_map executor built once (mirrors
    bass2jax.run_bass_via_pjrt, but cacheable across calls)."""
    if "runner" in _CACHE:
        return _CACHE["runner"]
    import jax
    from jax.sharding import Mesh, PartitionSpec
    from jax.experimental.shard_map import shard_map
    import concourse.mybir as mb
    from concourse.bass2jax import (
        _bass_exec_p, install_neuronx_cc_hook, partition_id_tensor)

    nc = _get_nc()
    install_neuronx_cc_hook()

    in_names, out_names, out_avals = [], [], []
    partition_name = (nc.partition_id_tensor.name
                      if nc.partition_id_tensor else None)
    for alloc in nc.m.functions[0].allocations:
        if not isinstance(alloc, mb.MemoryLocationSet):
            continue
        name = alloc.memorylocations[0].name
        if alloc.kind == "ExternalInput":
            if name != partition_name:
                in_names.append(name)
        elif alloc.kind == "ExternalOutput":
            out_names.append(name)
            out_avals.append(jax.core.ShapedArray(
                tuple(alloc.tensor_shape), mb.dt.np(alloc.dtype)))
    n_params = len(in_names)
    zero_outs = [np.zeros((8 * a.shape[0], *a.shape[1:]), a.dtype)
                 for a in out_avals]
    all_in_names = in_names + out_names
    if partition_name is not None:
        all_in_names = all_in_names + [partition_name]

    def _body(*args):
        operands = list(args)
        if partition_name is not None:
            operands.append(partition_id_tensor())
        return tuple(_bass_exec_p.bind(
            *operands,
            out_avals=tuple(out_avals),
            in_names=tuple(all_in_names),
            out_names=tuple(out_names),
            lowering_input_output_aliases=(),
            sim_require_finite=True,
            sim_require_nnan=True,
            nc=nc,
        ))

    devices = jax.devices()[:8]
    mesh = Mesh(np.asarray(devices), ("core",))
    nin = n_params + len(out_names)
    sharded = jax.jit(shard_map(
        _body, mesh=mesh,
        in_specs=(PartitionSpec("core"),) * nin,
        out_specs=(PartitionSpec("core"),) * len(out_names),
        check_rep=False))
    zeros_dev = [jax.device_put(z) for z in zero_outs]
    _CACHE["runner"] = (sharded, in_names, out_names, out_avals, zeros_dev)
    return _CACHE["runner"]


def run_cores(in_maps):
    """Run the 8-core program; returns list of per-core output dicts."""
    sharded, in_names, out_names, out_avals, zeros_dev = _get_runner()
    concat_in = [np.concatenate([m[name] for m in in_maps], axis=0)
                 for name in in_names]
    out_arrs = sharded(*concat_in, *zeros_dev)
    out_arrs = [np.asarray(o) for o in out_arrs]
    return [
        {name: out_arrs[i].reshape(8, *out_avals[i].shape)[c]
         for i, name in enumerate(out_names)}
        for c in range(8)
    ]


def build_in_maps(A, B, W1, b1, W2, b2):
    A = np.ascontiguousarray(np.asarray(A, dtype=np.float32))
    B = np.ascontiguousarray(np.asarray(B, dtype=np.float32))
    W1 = np.asarray(W1, dtype=np.float32)
    b1 = np.asarray(b1, dtype=np.float32)
    W2 = np.asarray(W2, dtype=np.float32)
    b2 = np.asarray(b2, dtype=np.float32)
    nbatch, seq, d = A.shape
    assert (nbatch, seq, d) == (4, N, D), (nbatch, seq, d)

    bf = ml_dtypes.bfloat16
    w1r = np.ascontiguousarray(W1).astype(bf)                     # [128, 256]
    w2r = np.ascontiguousarray(
        W2.reshape(2, 128, 2, 128).transpose(1, 0, 2, 3).reshape(128, 512)
    ).astype(bf)
    b1c = np.ascontiguousarray(b1.reshape(2, 128).T)              # [128, 2]
    b2c = np.ascontiguousarray(b2.reshape(2, 128).T)

    in_maps = []
    for core in range(8):
        b_i, half = divmod(core, 2)
        Ah = A[b_i, half * M:(half + 1) * M]                      # [2048, 128]
        Bf = B[b_i]                                               # [4096, 128]
        in_maps.append({
            "atb": np.ascontiguousarray(Ah.T).astype(bf),
            "btb": np.ascontiguousarray(Bf.T).astype(bf),
            "anr": np.ascontiguousarray(
                Ah.reshape(MC, 128, 128).transpose(1, 0, 2).reshape(128, M)
            ).astype(bf),
            "bnr": np.ascontiguousarray(
                Bf.reshape(NB, 128, 128).transpose(1, 0, 2).reshape(128, N)
            ).astype(bf),
            "w1": w1r, "w2": w2r, "b1": b1c, "b2": b2c,
        })
    return in_maps


def kernel(A, B, W1, b1, W2, b2):
    in_maps = build_in_maps(A, B, W1, b1, W2, b2)
    results = run_cores(in_maps)

    beta = np.empty((4, N, D), dtype=np.float32)
    alpha = np.empty((4, N, D), dtype=np.float32)
    for b_i in range(4):
        r0 = results[2 * b_i]
        r1 = results[2 * b_i + 1]
        for half, r in ((0, r0), (1, r1)):
            # rows[p, c] is the rowsum for m = c*128 + p
            rowv = r["rows"].T.reshape(1, M)
            beta[b_i, half * M:(half + 1) * M] = (r["betat"] / rowv).T
        num = r0["alphat"] + r1["alphat"]                          # [128, 4096]
        csum = r0["cols"] + r1["cols"]                             # [128, 32]
        # csum[p, j] corresponds to n = j*128 + p
        alpha[b_i] = (num / csum.T.reshape(1, N)).T.reshape(N, D)
    return beta, alpha


if __name__ == "__main__":
    rng = np.random.default_rng(0)
    A = rng.standard_normal((4, N, D)).astype(np.float32)
    B = rng.standard_normal((4, N, D)).astype(np.float32)
    s1, s2 = 1.0 / np.sqrt(D), 1.0 / np.sqrt(H)
    W1 = rng.uniform(-s1, s1, (D, H)).astype(np.float32)
    b1 = rng.uniform(-s1, s1, H).astype(np.float32)
    W2 = rng.uniform(-s2, s2, (H, H)).astype(np.float32)
    b2 = rng.uniform(-s2, s2, H).astype(np.float32)
    beta, alpha = kernel(A=A, B=B, W1=W1, b1=b1, W2=W2, b2=b2)
    print("beta", beta.shape, "alpha", alpha.shape)
